# revision 1
# baseline (speedup 1.0000x reference)
"""Trainium2 Bass kernel for nn_Decoder (scatter + gaussian conv + CTF filter).

Self-contained: hardcodes shapes/sharding for
  alignment (16,6), shifts (16,2), coords (500000,3), values (500000,),
  ctf (16,256,129) -> out (16,256,256) float32, 8 NeuronCores.

Sharding: pure data-parallel over the batch; each core handles 2 images.
Inside each core:
  - scatter: for each 128-point chunk build the two bilinear profile
    matrices; the value-weighted x-profile is scattered as 2-sparse
    rows by GPSIMD local_scatter, the y-profile is built densely as
    hat(iota - cy) = relu(1 - |iota - cy|): image 0 on the DVE with
    wide fused tensor ops, image 1 on the otherwise-idle ACT engine
    (Abs then Relu with per-partition bias).  The 256x256 image is
    accumulated in PSUM with PE matmuls yprof^T @ xw.
  - conv+FFT+CTF+iFFT: gaussian conv is folded into precomputed DFT
    matrices; the whole linear chain is fp32 matmuls + PE transposes.
"""
import sys
if '/opt/trn_rl_repo' not in sys.path:
    sys.path.insert(0, '/opt/trn_rl_repo')

import numpy as np
import concourse.bass as bass
import concourse.bacc as bacc
import concourse.mybir as mybir
from concourse.tile import TileContext
from concourse.bass_utils import run_bass_kernel_spmd

F16 = mybir.dt.float16
F32 = mybir.dt.float32
I16 = mybir.dt.int16
I32 = mybir.dt.int32
OP = mybir.AluOpType
ACT = mybir.ActivationFunctionType

XSIZE = 256
KX = 129
N_PTS = 500000
B_FULL = 16
N_CORES = 8
IMGS = 2                    # images per core
NCHUNK = 3920               # point chunks per image (128 pts each), padded
NPAD = NCHUNK * 128         # 501760 padded points
G = 7                       # chunks per scatter group / dst tile
NE = 256 * G                # 1792 dst columns per group
BODY_C = 56                 # chunks per For_i body (= 8 groups)
N_ITER = NCHUNK // BODY_C   # 70
NG = BODY_C // G            # 8 groups per body
ACT_G = 5                   # img1 y-groups built on the ACT engine per body


# ---------------------------------------------------------------- host mats
def _build_mats():
    n = XSIZE
    y = np.arange(n)
    ax = np.arange(5, dtype=np.float64) - 2.0
    g = np.exp(-(ax ** 2) / 2.0)
    gn = g / g.sum()
    Gm = np.zeros((n, n))
    for d in range(-2, 3):
        idx = np.arange(max(0, -d), min(n, n - d))
        Gm[idx, idx + d] = gn[d + 2]
    F = np.exp(-2j * np.pi * np.outer(y, y) / n)
    A = F @ Gm                                               # (256,256)
    Bh = np.exp(-2j * np.pi * np.outer(np.arange(KX), y) / n) @ Gm
    Bm = np.zeros((n, n), complex)
    Bm[:KX] = Bh                                             # kx zero-padded
    IFy = np.exp(+2j * np.pi * np.outer(y, y) / n) / n
    c = np.ones(KX)
    c[1:-1] = 2.0
    EXh = (np.exp(+2j * np.pi * np.outer(y, np.arange(KX)) / n) * c[None, :]) / n
    EX = np.zeros((n, n), complex)
    EX[:, :KX] = EXh

    def lhsT(M):  # (256,256) -> transposed, chunked (2,128,256) f32
        t = np.ascontiguousarray(M.T.reshape(2, 128, 256))
        return t.astype(np.float32)

    mats = {
        "ATr": lhsT(A.real), "ATi": lhsT(A.imag),
        "BrT": lhsT(Bm.real), "BiT": lhsT(Bm.imag), "nBiT": lhsT(-Bm.imag),
        "IFrT": lhsT(IFy.real), "IFiT": lhsT(IFy.imag), "nIFiT": lhsT(-IFy.imag),
        "EXrT": lhsT(EX.real), "nEXiT": lhsT(-EX.imag),
        "ident": np.eye(128, dtype=np.float32),
    }
    return mats


MAT_NAMES = ["ATr", "ATi", "BrT", "BiT", "nBiT", "IFrT", "IFiT", "nIFiT",
             "EXrT", "nEXiT"]

# sc columns (per image, 16 cols): 0-2 x row coeffs, 3-5 y row coeffs,
# 6 y const (128 - sy), 8-10 negated y coeffs, 11 negated y const
C_A, C_B, C_CY, C_NB, C_NCY = 0, 3, 6, 8, 11


# ---------------------------------------------------------------- bass build
def _build_nc():
    nc = bacc.Bacc()
    xt_in = nc.declare_dram_parameter("xt", [128, NCHUNK], F32, isOutput=False)
    yt_in = nc.declare_dram_parameter("yt", [128, NCHUNK], F32, isOutput=False)
    zt_in = nc.declare_dram_parameter("zt", [128, NCHUNK], F32, isOutput=False)
    vt_in = nc.declare_dram_parameter("vt", [128, NCHUNK], F32, isOutput=False)
    sc_in = nc.declare_dram_parameter("sc", [128, IMGS * 16], F32,
                                      isOutput=False)
    xoffc_in = nc.declare_dram_parameter("xoffc", [128, IMGS * BODY_C], F32,
                                         isOutput=False)
    yoffc_in = nc.declare_dram_parameter("yoffc", [128, BODY_C], F32,
                                         isOutput=False)
    xoffy_in = nc.declare_dram_parameter("xoffy", [128, BODY_C], F32,
                                         isOutput=False)
    iota_in = nc.declare_dram_parameter("iota", [128, 256], F32,
                                        isOutput=False)
    ctf_in = nc.declare_dram_parameter("ctfT", [IMGS, 256, 256], F32,
                                       isOutput=False)
    mat_in = {m: nc.declare_dram_parameter(m, [2, 128, 256], F32,
                                           isOutput=False)
              for m in MAT_NAMES}
    id_in = nc.declare_dram_parameter("ident", [128, 128], F32, isOutput=False)
    out_d = nc.declare_dram_parameter("out", [IMGS, 256, 256], F32,
                                      isOutput=True)

    with TileContext(nc) as tc:
        with tc.tile_pool(name="inp", bufs=1) as inp, \
             tc.tile_pool(name="mat", bufs=1) as matp, \
             tc.tile_pool(name="prep", bufs=2) as prep, \
             tc.tile_pool(name="dstp", bufs=2) as dstp, \
             tc.tile_pool(name="work", bufs=1) as work, \
             tc.tile_pool(name="accp", bufs=1, space="PSUM") as accp, \
             tc.tile_pool(name="eps", bufs=4, space="PSUM") as eps:

            # ---------------- load inputs ----------------
            xt = inp.tile([128, NCHUNK], F32)
            yt = inp.tile([128, NCHUNK], F32)
            zt = inp.tile([128, NCHUNK], F32)
            vt = inp.tile([128, NCHUNK], F32)
            nc.sync.dma_start(xt[:], xt_in[:])
            nc.sync.dma_start(yt[:], yt_in[:])
            nc.sync.dma_start(zt[:], zt_in[:])
            nc.sync.dma_start(vt[:], vt_in[:])

            sc = inp.tile([128, IMGS * 16], F32)
            nc.sync.dma_start(sc[:], sc_in[:])
            xoffc = inp.tile([128, IMGS * BODY_C], F32)
            nc.sync.dma_start(xoffc[:], xoffc_in[:])
            yoffc = inp.tile([128, BODY_C], F32)
            nc.sync.dma_start(yoffc[:], yoffc_in[:])
            xoffy = inp.tile([128, BODY_C], F32)
            nc.sync.dma_start(xoffy[:], xoffy_in[:])
            # iota passes through Abs+Relu (exact on >=0 values) so the ACT
            # function table is loaded before the loop; a bare dummy
            # activation gets sunk past the loop and the framework would
            # then reload the table every iteration.
            iota_raw = inp.tile([128, 256], F32)
            nc.sync.dma_start(iota_raw[:], iota_in[:])
            iota = inp.tile([128, 256], F32)
            nc.scalar.activation(iota[:], iota_raw[:], ACT.Abs,
                                 bias=0.0, scale=1.0)
            nc.scalar.activation(iota[:], iota[:], ACT.Relu,
                                 bias=0.0, scale=1.0)

            mats = {}
            for m in MAT_NAMES:
                t0 = matp.tile([128, 256], F32, tag=f"{m}0")
                t1 = matp.tile([128, 256], F32, tag=f"{m}1")
                nc.sync.dma_start(t0[:], mat_in[m][0])
                nc.sync.dma_start(t1[:], mat_in[m][1])
                mats[m] = (t0, t1)
            ident = matp.tile([128, 128], F32)
            nc.sync.dma_start(ident[:], id_in[:])
            ctfs = []
            for b in range(IMGS):
                c0 = matp.tile([128, 256], F32, tag=f"ctf{b}0")
                c1 = matp.tile([128, 256], F32, tag=f"ctf{b}1")
                nc.sync.dma_start(c0[:], ctf_in[b, 0:128, :])
                nc.sync.dma_start(c1[:], ctf_in[b, 128:256, :])
                ctfs.append((c0, c1))

            zero16 = inp.tile([128, 256], F16)
            nc.vector.memset(zero16[:], 0.0)

            # Dummy scatter so the GPSIMD local_scatter ucode library is
            # loaded before the loop; otherwise the framework reloads it
            # every iteration (an 11us GPSIMD drain per reload).
            dum_idx = inp.tile([128, 2], I16)
            nc.vector.memset(dum_idx[:], -1.0)
            dum_dat = inp.tile([128, 2], F16)
            nc.vector.memset(dum_dat[:], 0.0)
            dum_dst = inp.tile([128, 2], F16)
            nc.gpsimd.local_scatter(dum_dst[:], dum_dat[:], dum_idx[:],
                                    channels=128, num_elems=2, num_idxs=2)

            # ---------------- PSUM accumulators ----------------
            acc = [[accp.tile([128, 256], F32, tag=f"acc{b}{h}",
                               name=f"acc_{b}_{h}")
                    for h in range(2)] for b in range(IMGS)]
            for b in range(IMGS):
                for h in range(2):
                    nc.tensor.matmul(acc[b][h][:], zero16[:, 0:128],
                                     zero16[:], start=True, stop=False)

            # ---------------- main scatter loop ----------------
            def proj(dst, base, c0, c1, c2, last_tile=None, last_scalar=None):
                """dst = xt*c0 + yt*c1 + zt*c2 + (tile or scalar) over BODY_C
                chunks at `base`.  c* are [128,1] scalar APs."""
                t0 = prep.tile([128, BODY_C], F32, tag="p_t0")
                if last_tile is not None:
                    nc.vector.scalar_tensor_tensor(
                        t0[:], xt[:, bass.DynSlice(base, BODY_C)], c0,
                        last_tile, op0=OP.mult, op1=OP.add)
                else:
                    nc.vector.tensor_scalar(
                        t0[:], xt[:, bass.DynSlice(base, BODY_C)], c0,
                        last_scalar, op0=OP.mult, op1=OP.add)
                t1 = prep.tile([128, BODY_C], F32, tag="p_t1")
                nc.vector.scalar_tensor_tensor(
                    t1[:], yt[:, bass.DynSlice(base, BODY_C)], c1, t0[:],
                    op0=OP.mult, op1=OP.add)
                nc.vector.scalar_tensor_tensor(
                    dst[:], zt[:, bass.DynSlice(base, BODY_C)], c2, t1[:],
                    op0=OP.mult, op1=OP.add)

            def build_idxdat(cof, base, lo, width, tag, weighted):
                """Floor/frac + scatter idx/dat over chunk cols [lo,lo+width)
                of the offset-included coords `cof`.  Returns (idx, dat)
                [128,width,2] tiles; dat is v-weighted when `weighted`."""
                cs = cof[:, lo:lo + width]
                ii = prep.tile([128, width], I32, tag=f"ii{tag}")
                nc.vector.tensor_copy(ii[:], cs)
                dd = prep.tile([128, width], F32, tag=f"dd{tag}")
                nc.vector.tensor_copy(dd[:], ii[:])
                gt = prep.tile([128, width], F32, tag=f"gt{tag}")
                nc.vector.tensor_tensor(gt[:], dd[:], cs, op=OP.is_gt)
                fl = prep.tile([128, width], F32, tag=f"fl{tag}")
                nc.vector.tensor_tensor(fl[:], dd[:], gt[:], op=OP.subtract)
                fr = prep.tile([128, width], F32, tag=f"fr{tag}")
                nc.vector.tensor_tensor(fr[:], cs, fl[:], op=OP.subtract)
                idx_t = prep.tile([128, width, 2], I16, tag=f"idx{tag}")
                nc.vector.tensor_copy(idx_t[:, :, 0], fl[:])
                nc.vector.tensor_scalar(idx_t[:, :, 1], fl[:], 1.0, None,
                                        op0=OP.add)
                dat_t = prep.tile([128, width, 2], F16, tag=f"dat{tag}")
                if weighted:
                    vs = vt[:, bass.DynSlice(base + lo, width)]
                    vfx = prep.tile([128, width], F32, tag=f"vfx{tag}")
                    nc.vector.tensor_tensor(vfx[:], vs, fr[:], op=OP.mult)
                    nc.vector.tensor_tensor(dat_t[:, :, 0], vs, vfx[:],
                                            op=OP.subtract)
                    nc.vector.tensor_copy(dat_t[:, :, 1], vfx[:])
                else:
                    nc.vector.tensor_scalar(dat_t[:, :, 0], fr[:], -1.0, 1.0,
                                            op0=OP.mult, op1=OP.add)
                    nc.vector.tensor_copy(dat_t[:, :, 1], fr[:])
                return idx_t, dat_t

            with tc.For_i(0, N_ITER, 1) as it:
                base = it * BODY_C
                xidx, xdat, yidx, ydat = {}, {}, {}, {}
                ncy1 = None
                for b in range(IMGS):
                    o = 16 * b
                    # ---- x-axis prep: cx + slot offset + const
                    cxo = prep.tile([128, BODY_C], F32, tag=f"cxo{b}")
                    proj(cxo, base, sc[:, o + C_A:o + C_A + 1],
                         sc[:, o + C_A + 1:o + C_A + 2],
                         sc[:, o + C_A + 2:o + C_A + 3],
                         last_tile=xoffc[:, BODY_C * b:BODY_C * (b + 1)])
                    xidx[b], xdat[b] = build_idxdat(
                        cxo, base, 0, BODY_C, f"x{b}", weighted=True)
                    # ---- y-axis prep
                    if b == 0:
                        cyo = prep.tile([128, BODY_C], F32, tag="cyo0")
                        proj(cyo, base, sc[:, o + C_B:o + C_B + 1],
                             sc[:, o + C_B + 1:o + C_B + 2],
                             sc[:, o + C_B + 2:o + C_B + 3],
                             last_tile=yoffc[:])
                        yidx[b], ydat[b] = build_idxdat(
                            cyo, base, 0, BODY_C, "y0", weighted=False)
                    else:
                        cyb = prep.tile([128, BODY_C], F32, tag="cyb1")
                        proj(cyb, base, sc[:, o + C_B:o + C_B + 1],
                             sc[:, o + C_B + 1:o + C_B + 2],
                             sc[:, o + C_B + 2:o + C_B + 3],
                             last_scalar=sc[:, o + C_CY:o + C_CY + 1])
                        ncy1 = prep.tile([128, BODY_C], F32, tag="ncy1")
                        nc.vector.tensor_scalar(ncy1[:], cyb[:], -1.0, None,
                                                op0=OP.mult)
                        if ACT_G < NG:
                            cyo1 = prep.tile([128, BODY_C], F32, tag="cyo1")
                            nc.vector.tensor_tensor(cyo1[:], cyb[:],
                                                    xoffy[:], op=OP.add)
                            yidx[b], ydat[b] = build_idxdat(
                                cyo1, base, G * ACT_G, BODY_C - G * ACT_G,
                                "y1", weighted=False)

                for g in range(NG):
                    for b in range(IMGS):
                        # x-profile: GPSIMD 2-sparse scatter
                        xd = dstp.tile([128, NE], F16, tag=f"xd{b}")
                        nc.gpsimd.local_scatter(
                            xd[:], xdat[b][:, G * g:G * (g + 1), :],
                            xidx[b][:, G * g:G * (g + 1), :],
                            channels=128, num_elems=NE, num_idxs=2 * G)
                        # y-profile
                        yd = dstp.tile([128, NE], F16, tag=f"yd{b}")
                        if b == 1 and g < ACT_G:
                            # ACT: u=Abs(iota-cy), w=Relu(-u+1), per chunk
                            for s in range(G):
                                c = G * g + s
                                u2 = prep.tile([128, 256], F16,
                                               tag=f"act_u{s % 2}")
                                nc.scalar.activation(
                                    u2[:], iota[:, 0:256], ACT.Abs,
                                    bias=ncy1[:, c:c + 1], scale=1.0)
                                nc.scalar.activation(
                                    yd[:, 256 * s:256 * (s + 1)], u2[:],
                                    ACT.Relu, bias=1.0, scale=-1.0)
                        else:
                            gl = g if b == 0 else g - ACT_G
                            nc.gpsimd.local_scatter(
                                yd[:], ydat[b][:, G * gl:G * (gl + 1), :],
                                yidx[b][:, G * gl:G * (gl + 1), :],
                                channels=128, num_elems=NE, num_idxs=2 * G)
                        # matmuls for this group
                        for s in range(G):
                            rhs = xd[:, 256 * s:256 * (s + 1)]
                            for h in range(2):
                                lhsT = yd[:, 256 * s + 128 * h:
                                          256 * s + 128 * (h + 1)]
                                nc.tensor.matmul(acc[b][h][:], lhsT, rhs,
                                                 start=False, stop=False)

            for b in range(IMGS):
                for h in range(2):
                    nc.tensor.matmul(acc[b][h][:], zero16[:, 0:128],
                                     zero16[:], start=False, stop=True)

            # ---------------- epilogue: conv+FFT+CTF+iFFT ----------------
            def mm_pair(out_ps, lT, rhs_tiles, extra=None, first=True):
                """out_ps += sum_kc lT[kc]^T @ rhs_tiles[kc] (+ extra pair)."""
                ops = []
                for kc in range(2):
                    ops.append((lT[kc], rhs_tiles[kc]))
                if extra is not None:
                    lT2, rhs2 = extra
                    for kc in range(2):
                        ops.append((lT2[kc], rhs2[kc]))
                for j, (lt, rh) in enumerate(ops):
                    nc.tensor.matmul(out_ps[:], lt, rh,
                                     start=(first and j == 0),
                                     stop=(j == len(ops) - 1))

            def transpose_mat(src_tiles, tag):
                """src: 2 SBUF tiles (128,256) = (256,256) matrix -> transposed."""
                dst = [work.tile([128, 256], F32, tag=f"{tag}{m}",
                                 name=f"tr_{tag}_{m}")
                       for m in range(2)]
                for a in range(2):
                    for bcol in range(2):
                        pt = eps.tile([128, 128], F32, tag="ep")
                        nc.tensor.transpose(
                            pt[:], src_tiles[a][:, 128 * bcol:128 * (bcol + 1)],
                            ident[:])
                        nc.vector.tensor_copy(
                            dst[bcol][:, 128 * a:128 * (a + 1)], pt[:])
                return dst

            def cmul_stage(lr, li, nli, rhs_r, rhs_i, tag):
                """Complex matmul stage: returns (out_r, out_i) SBUF tiles.

                out_r = lr^T@rhs_r + nli^T@rhs_i ; out_i = lr^T@rhs_i + li^T@rhs_r
                Each output is 2 M-half tiles (128,256).
                """
                outr, outi = [], []
                for m in range(2):
                    lrm = [lr[kc][:, 128 * m:128 * (m + 1)] for kc in range(2)]
                    lim = [li[kc][:, 128 * m:128 * (m + 1)] for kc in range(2)]
                    nlim = [nli[kc][:, 128 * m:128 * (m + 1)] for kc in range(2)]
                    pr = eps.tile([128, 256], F32, tag="ep")
                    mm_pair(pr, lrm, rhs_r, extra=(nlim, rhs_i))
                    tr = work.tile([128, 256], F32, tag=f"{tag}r{m}")
                    nc.vector.tensor_copy(tr[:], pr[:])
                    outr.append(tr)
                    pi = eps.tile([128, 256], F32, tag="ep")
                    mm_pair(pi, lrm, rhs_i, extra=(lim, rhs_r))
                    ti = work.tile([128, 256], F32, tag=f"{tag}i{m}")
                    nc.vector.tensor_copy(ti[:], pi[:])
                    outi.append(ti)
                return outr, outi

            for b in range(IMGS):
                img_sb = [work.tile([128, 256], F32, tag=f"img{h}",
                                    name=f"img_sb_{h}")
                          for h in range(2)]
                for h in range(2):
                    nc.vector.tensor_copy(img_sb[h][:], acc[b][h][:])
                # U = A @ img
                Ur, Ui = [], []
                for m in range(2):
                    for part, lst in (("r", Ur), ("i", Ui)):
                        mat = mats["ATr" if part == "r" else "ATi"]
                        ps = eps.tile([128, 256], F32, tag="ep")
                        mm_pair(ps, [mat[kc][:, 128 * m:128 * (m + 1)]
                                     for kc in range(2)], img_sb)
                        t = work.tile([128, 256], F32, tag=f"U{part}{m}")
                        nc.vector.tensor_copy(t[:], ps[:])
                        lst.append(t)
                UTr = transpose_mat(Ur, "UTr")
                UTi = transpose_mat(Ui, "UTi")
                # ST = B @ UT ; then ctf
                STr, STi = cmul_stage(mats["BrT"], mats["BiT"], mats["nBiT"],
                                      UTr, UTi, "ST")
                Spr, Spi = [], []
                for m in range(2):
                    tr = work.tile([128, 256], F32, tag=f"Spr{m}")
                    nc.vector.tensor_tensor(tr[:], STr[m][:], ctfs[b][m][:],
                                            op=OP.mult)
                    Spr.append(tr)
                    ti = work.tile([128, 256], F32, tag=f"Spi{m}")
                    nc.vector.tensor_tensor(ti[:], STi[m][:], ctfs[b][m][:],
                                            op=OP.mult)
                    Spi.append(ti)
                SpTr = transpose_mat(Spr, "SpTr")
                SpTi = transpose_mat(Spi, "SpTi")
                # W = IFy @ Sp
                Wr, Wi = cmul_stage(mats["IFrT"], mats["IFiT"], mats["nIFiT"],
                                    SpTr, SpTi, "W")
                WTr = transpose_mat(Wr, "WTr")
                WTi = transpose_mat(Wi, "WTi")
                # outT = Re(EX @ WT)
                for m in range(2):
                    po = eps.tile([128, 256], F32, tag="ep")
                    mm_pair(po, [mats["EXrT"][kc][:, 128 * m:128 * (m + 1)]
                                 for kc in range(2)], WTr,
                            extra=([mats["nEXiT"][kc][:, 128 * m:128 * (m + 1)]
                                    for kc in range(2)], WTi))
                    ot = work.tile([128, 256], F32, tag=f"outT{m}")
                    nc.vector.tensor_copy(ot[:], po[:])
                    nc.sync.dma_start(out_d[b, 128 * m:128 * (m + 1), :], ot[:])
    nc.finalize()
    return nc


_NC_CACHE = None
_TRACE = False
_TMPDIR = None
_LAST_RES = None


def _get_nc():
    global _NC_CACHE
    if _NC_CACHE is None:
        _NC_CACHE = _build_nc()
    return _NC_CACHE


# ---------------------------------------------------------------- host entry
def kernel(alignment, shifts, coords, values, ctf):
    alignment = np.asarray(alignment, np.float32)
    shifts = np.asarray(shifts, np.float32)
    coords = np.asarray(coords, np.float32)
    values = np.asarray(values, np.float32)
    ctf = np.asarray(ctf, np.float32)

    # pad points; pad coords with a copy of point 0 (in range), v=0
    cpad = np.empty((NPAD, 3), np.float32)
    cpad[:N_PTS] = coords
    cpad[N_PTS:] = coords[0]
    vpad = np.zeros((NPAD,), np.float32)
    vpad[:N_PTS] = values
    fx = np.ascontiguousarray(cpad[:, 0].reshape(128, NCHUNK))
    fy = np.ascontiguousarray(cpad[:, 1].reshape(128, NCHUNK))
    fz = np.ascontiguousarray(cpad[:, 2].reshape(128, NCHUNK))
    fv = np.ascontiguousarray(vpad.reshape(128, NCHUNK))

    mats = _build_mats()
    iota = np.ascontiguousarray(
        np.arange(256, dtype=np.float32)[None, :].repeat(128, axis=0))
    xoffy = np.ascontiguousarray(
        (256.0 * (np.arange(BODY_C) % G)).astype(np.float32)[None, :]
        .repeat(128, axis=0))

    in_maps = []
    for c in range(N_CORES):
        b0 = IMGS * c
        sc = np.zeros((128, IMGS * 16), np.float32)
        xoffc = np.zeros((128, IMGS * BODY_C), np.float32)
        for b in range(IMGS):
            al = alignment[b0 + b]
            o = 16 * b
            sc[:, o + C_A:o + C_A + 3] = al[0:3]
            sc[:, o + C_B:o + C_B + 3] = al[3:6]
            sc[:, o + C_CY] = 128.0 - shifts[b0 + b, 1]
            sc[:, o + C_NB:o + C_NB + 3] = -al[3:6]
            sc[:, o + C_NCY] = shifts[b0 + b, 1] - 128.0
            xoffc[:, BODY_C * b:BODY_C * (b + 1)] = (
                256.0 * (np.arange(BODY_C) % G)
                + 128.0 - shifts[b0 + b, 0]).astype(np.float32)
        yoffc = np.ascontiguousarray(
            (256.0 * (np.arange(BODY_C) % G)
             + 128.0 - shifts[b0 + 0, 1]).astype(np.float32)[None, :]
            .repeat(128, axis=0))
        ctfT = np.zeros((IMGS, 256, 256), np.float32)
        ctfT[:, :KX, :] = np.transpose(ctf[b0:b0 + IMGS], (0, 2, 1))
        m = {"xt": fx, "yt": fy, "zt": fz, "vt": fv,
             "sc": sc, "xoffc": xoffc, "yoffc": yoffc, "xoffy": xoffy,
             "iota": iota, "ctfT": ctfT,
             "ident": mats["ident"]}
        for name in MAT_NAMES:
            m[name] = mats[name]
        in_maps.append(m)

    nc = _get_nc()
    res = run_bass_kernel_spmd(nc, in_maps, list(range(N_CORES)),
                               trace=_TRACE, tmpdir=_TMPDIR)
    global _LAST_RES
    _LAST_RES = res
    out = np.empty((B_FULL, 256, 256), np.float32)
    for c in range(N_CORES):
        o = res.results[c]["out"]          # (2, 256, 256) x-major
        for b in range(IMGS):
            out[IMGS * c + b] = o[b].T
    return out


if __name__ == "__main__":
    d = np.load("/root/problem/work/ref_cache.npz")
    ins = {k: d[k] for k in ["alignment", "shifts", "coords", "values", "ctf"]}
    o = kernel(**ins)
    ref = d["ref"]
    err = np.abs(o - ref).max() / np.abs(ref).max()
    print("rel err:", err)

